# revision 1
# baseline (speedup 1.0000x reference)
"""Trainium2 Bass kernel for nn_AttentionBlock (column-softmax causal attention).

Reference computation (B=4, S=4096, D=128, K=64, V=128):
    Q = x @ Wq.T + bq            [B,S,64]
    Km = x @ Wk.T + bk           [B,S,64]
    Vm = x @ Wv.T + bv           [B,S,128]
    s  = Q @ Km.T / 8            [B,S,S], causal mask j>q -> -1e9
    p  = softmax(s, axis=1)      (softmax over the QUERY axis -- column softmax)
    att = p @ Vm                 [B,S,128]
    out = concat(x, att, dim=2)  [B,S,256]

Key observation: with ST = s.T (layout [j, q]) the softmax denominator
l[j] = sum_q exp(ST[j, q]) is a free-dim reduction, so
att[q] = sum_j exp(ST[j,q]) * (Vm[j]/l[j]) -- a flash-style two-phase kernel
with NO max subtraction needed (scores are O(+-20), exp is safe in fp32).

Sharding (8 cores): core c -> batch b = c//2, j-tile parity p = c%2.
Each core computes l[j] and the PV partial sum for its 16 j-tiles
(j-tile J = 2*i + p), over all q. Host adds the two partials per batch.
All parity differences are data-driven (xkv row gather + additive mask
input), so one SPMD program serves all 8 cores.

Performance structure per core:
  - QK score matmuls run as f32r with row-PAIR packing: rows 2r / 2r+1 use
    the two 64-partition halves of the PE array concurrently (KT/QT are
    duplicated into both partition halves so tile_position auto-derives).
  - exp runs on ACT with fused per-partition accumulation (accum_out = l).
  - PV runs transposed: attT[v, q] = sum_j Vp[j,v] * E[j,q] with N=512
    moving operands (bf16), then PE-transposes back to [q, v] tiles.
"""

import numpy as np

B, S, D = 4, 4096, 128
KD, VD = 64, 128
P = 128
NCORES = 8
JT = 16           # local j-tiles per core
NT = S // P       # 32 global q/j tiles
CHUNK = 1024      # ACT exp chunk width (PSUM cols)

QK_F32R = True

ROW_W = [S - 2 * i * P for i in range(JT)]          # E row widths
EOFF = [0] * JT
for _i in range(1, JT):
    EOFF[_i] = EOFF[_i - 1] + ROW_W[_i - 1]
ECOLS = EOFF[-1] + ROW_W[-1]                        # 34816

_CACHE = {}


def _build_program():
    from contextlib import ExitStack

    from concourse import bacc, mybir
    from concourse import tile as tile_mod

    dt = mybir.dt
    f32, bf16 = dt.float32, dt.bfloat16
    Alu = mybir.AluOpType
    ActF = mybir.ActivationFunctionType

    nc = bacc.Bacc(
        "TRN2", target_bir_lowering=False, debug=False, num_devices=NCORES
    )

    # Operand tiles of f32r matmuls must be PRODUCED as float32r (the BIR
    # verifier requires the producing instruction to round); they are
    # written by DVE ops (which round) or DMA'd in as float32r directly.
    mmdt = dt.float32r if QK_F32R else f32

    # Host supplies x^T / xkv^T / W^T (pure layout prep) so the kernel
    # spends no PE/ACT/DVE time transposing, and the DMAs are contiguous.
    xt_d = nc.dram_tensor("xt", [P, S], mmdt, kind="ExternalInput").ap()
    xkvt_d = nc.dram_tensor("xkvt", [P, JT * P], mmdt, kind="ExternalInput").ap()
    # all small per-core inputs packed into one tensor (one DMA issue):
    # wqt[0:128] wkt[128:256] wvt[256:384] bq[384] bk[385] bv[386] mrow[387:643]
    small_d = nc.dram_tensor("small", [P, 643], mmdt, kind="ExternalInput").ap()
    att_d = nc.dram_tensor("att", [S, VD], f32, kind="ExternalOutput").ap()

    with tile_mod.TileContext(nc) as tc, ExitStack() as ctx:
        persist = ctx.enter_context(tc.tile_pool(name="persist", bufs=1))

        xT = persist.tile([P, S], mmdt)            # [d, q]
        xkvT = persist.tile([P, JT * P], mmdt)     # [d, local j]
        # QT/KT live in BOTH partition halves (rows 0-63 == rows 64-127) so
        # QK row pairs can use tile_position (0,0)/(64,0) concurrently.
        QT = persist.tile([P, S], mmdt)            # [k(dup), q]
        KTl = persist.tile([P, JT * P], mmdt)      # [k(dup), local j]
        V_sb = persist.tile([P, JT, VD], f32)      # [local j, v]
        Vp_sb = persist.tile([P, JT, VD], bf16)    # V / l
        E_all = persist.tile([P, ECOLS], bf16)     # exp(scores.T) rows
        l_all = persist.tile([P, JT], f32)
        linv = persist.tile([P, JT], f32)
        VT_sb = persist.tile([P, JT * P], f32)     # [v, local j]
        small_sb = persist.tile([P, 643], mmdt)
        ident = persist.tile([P, P], f32)
        WqT = small_sb[:, 0:128]
        WkT = small_sb[:, 128:256]
        WvT = small_sb[:, 256:384]
        bq_sb = small_sb[:, 384:385].bitcast(f32)
        bk_sb = small_sb[:, 385:386].bitcast(f32)
        bv_sb = small_sb[:, 386:387].bitcast(f32)
        mrow_sb = small_sb[:, 387:643].bitcast(f32)

        # ---- input DMAs: 5 issues total (SWDGE descriptor-gen is ~0.6us
        # of serial sequencer time PER dma_start), split across two
        # sequencers; high halves first (reverse pair order needs them).
        nc.gpsimd.dma_start(out=small_sb, in_=small_d)
        nc.gpsimd.dma_start(
            out=xkvT[:, 1024:2048], in_=xkvt_d[:, 1024:2048]
        )
        nc.gpsimd.dma_start(out=xkvT[:, 0:1024], in_=xkvt_d[:, 0:1024])
        nc.sync.dma_start(out=xT[:, 2048:4096], in_=xt_d[:, 2048:4096])
        nc.sync.dma_start(out=xT[:, 0:2048], in_=xt_d[:, 0:2048])

        # identity for PE transposes
        nc.gpsimd.memset(ident, 0.0)
        nc.gpsimd.affine_select(
            out=ident,
            in_=ident,
            compare_op=Alu.not_equal,
            fill=1.0,
            base=0,
            pattern=[[-1, P]],
            channel_multiplier=1,
        )

        # ---- phase A/B: row pairs in REVERSE order, each preceded only by
        # the KT/QT projection chunks it needs (so the PE FIFO never blocks
        # on input DMA for data a later pair needs), with the V path spread
        # through the (ACT-bound) pair phase.
        with ExitStack() as pha:
            prj = pha.enter_context(
                tc.tile_pool(name="prj_psum", bufs=2, space="PSUM")
            )
            rowp = pha.enter_context(
                tc.tile_pool(name="row_psum", bufs=3, space="PSUM")
            )
            lpp = pha.enter_context(tc.tile_pool(name="lparts", bufs=8))

            def emit_kt_chunk(c):
                ps = prj.tile([P, 512], f32, tag="prj", name=f"kt_{c}")
                nc.tensor.matmul(
                    ps,
                    lhsT=WkT,
                    rhs=xkvT[:, c * 512 : (c + 1) * 512],
                    start=True,
                    stop=True,
                )
                nc.vector.tensor_scalar(
                    out=KTl[:, c * 512 : (c + 1) * 512],
                    in0=ps,
                    scalar1=bk_sb,
                    scalar2=None,
                    op0=Alu.add,
                )

            def emit_qt_chunk(c):
                ps = prj.tile([P, 512], f32, tag="prj", name=f"qt_{c}")
                nc.tensor.matmul(
                    ps,
                    lhsT=WqT,
                    rhs=xT[:, c * 512 : (c + 1) * 512],
                    start=True,
                    stop=True,
                )
                nc.vector.tensor_scalar(
                    out=QT[:, c * 512 : (c + 1) * 512],
                    in0=ps,
                    scalar1=bq_sb,
                    scalar2=None,
                    op0=Alu.add,
                )

            def emit_v_group(g):
                # VT chunk g -> V tiles [j, v] for rows 4g..4g+3, then
                # V' = V/l (their l is complete once pair 2g is done).
                ps = prj.tile([P, 512], f32, tag="prj", name=f"vt_{g}")
                nc.tensor.matmul(
                    ps,
                    lhsT=WvT,
                    rhs=xkvT[:, g * 512 : (g + 1) * 512],
                    start=True,
                    stop=True,
                )
                nc.vector.tensor_scalar(
                    out=VT_sb[:, g * 512 : (g + 1) * 512],
                    in0=ps,
                    scalar1=bv_sb,
                    scalar2=None,
                    op0=Alu.add,
                )
                pst = prj.tile([P, 4, P], f32, tag="prj", name=f"vtp_{g}")
                for k in range(4):
                    i = g * 4 + k
                    nc.tensor.transpose(
                        pst[:, k, :], VT_sb[:, i * P : (i + 1) * P], ident
                    )
                nc.vector.tensor_copy(
                    V_sb[:, g * 4 : (g + 1) * 4, :].rearrange(
                        "p a b -> p (a b)"
                    ),
                    pst.rearrange("p a b -> p (a b)"),
                )
                for i in range(4 * g, 4 * g + 4):
                    nc.vector.reciprocal(
                        linv[:, i : i + 1], l_all[:, i : i + 1]
                    )
                    nc.vector.tensor_scalar(
                        out=Vp_sb[:, i, :],
                        in0=V_sb[:, i, :],
                        scalar1=linv[:, i : i + 1],
                        scalar2=None,
                        op0=Alu.mult,
                    )

            def emit_row_pair(r):
                # rows 2r (partition half 0) and 2r+1 (half 64), MMs
                # interleaved at 512-slice granularity so the PE overlaps
                # them in opposite array halves.
                state = {}
                for i in (2 * r, 2 * r + 1):
                    q0 = 256 * i
                    w = ROW_W[i]
                    chunks = [
                        (q0 + c * CHUNK, min(CHUNK, w - c * CHUNK))
                        for c in range((w + CHUNK - 1) // CHUNK)
                    ]
                    slices = []
                    for ci, (off, cw) in enumerate(chunks):
                        for s0 in range(0, cw, 512):
                            slices.append((ci, off, cw, s0, min(512, cw - s0)))
                    state[i] = {"chunks": chunks, "slices": slices, "ps": {}}

                def finish_chunk(i, ci, cw):
                    st = state[i]
                    ps = st["ps"][ci]
                    if ci == 0:
                        nc.vector.tensor_add(
                            ps[:, : 2 * P], ps[:, : 2 * P], mrow_sb
                        )
                    lp = lpp.tile([P, 1], f32, tag="lp", name=f"lp_{i}_{ci}")
                    ecol = EOFF[i] + ci * CHUNK
                    nc.scalar.activation(
                        out=E_all[:, ecol : ecol + cw],
                        in_=ps[:, :cw],
                        func=ActF.Exp,
                        accum_out=lp,
                    )
                    if ci == 0:
                        nc.vector.tensor_copy(l_all[:, i : i + 1], lp)
                    else:
                        nc.vector.tensor_add(
                            l_all[:, i : i + 1], l_all[:, i : i + 1], lp
                        )

                nslice = max(len(state[i]["slices"]) for i in state)
                for k in range(nslice):
                    for idx, i in enumerate((2 * r, 2 * r + 1)):
                        st = state[i]
                        if k >= len(st["slices"]):
                            continue
                        ci, off, cw, s0, sw = st["slices"][k]
                        if ci not in st["ps"]:
                            st["ps"][ci] = rowp.tile(
                                [P, CHUNK], f32, tag="st", name=f"st_{i}_{ci}"
                            )
                        base = KD * idx
                        nc.tensor.matmul(
                            st["ps"][ci][:, s0 : s0 + sw],
                            lhsT=KTl[base : base + KD, i * P : (i + 1) * P],
                            rhs=QT[base : base + KD, off + s0 : off + s0 + sw],
                            start=True,
                            stop=True,
                        )
                        if s0 + sw == cw:
                            finish_chunk(i, ci, cw)

            kt_done = set()
            for r in reversed(range(8)):
                if r // 2 not in kt_done:
                    kt_done.add(r // 2)
                    emit_kt_chunk(r // 2)
                emit_qt_chunk(r)
                emit_row_pair(r)
                if r % 2 == 0:
                    emit_v_group(r // 2)

        # ---- phase C: PV block ------------------------------------------
        with ExitStack() as phc:
            attp = phc.enter_context(
                tc.tile_pool(name="att_psum", bufs=4, space="PSUM")
            )
            tsbp = phc.enter_context(tc.tile_pool(name="attT_sb", bufs=2))
            sbo = phc.enter_context(tc.tile_pool(name="att_sb", bufs=2))

            def emit_pv_chunk(c):
                # attT[v, q] for q in [512c, 512c+512): rows i <= 2c full
                # coverage (N=512), row 2c+1 covers the second half (N=256).
                attT = attp.tile([P, 4, P], f32, tag="attT", name=f"attT_{c}")
                aflat = attT.rearrange("p a b -> p (a b)")
                for i in range(2 * c + 1):
                    ecol = EOFF[i] + 512 * c - 256 * i
                    nc.tensor.matmul(
                        aflat,
                        lhsT=Vp_sb[:, i, :],
                        rhs=E_all[:, ecol : ecol + 512],
                        start=(i == 0),
                        stop=False,
                    )
                i2 = 2 * c + 1
                nc.tensor.matmul(
                    aflat[:, 256:512],
                    lhsT=Vp_sb[:, i2, :],
                    rhs=E_all[:, EOFF[i2] : EOFF[i2] + 256],
                    start=False,
                    stop=True,
                )
                tsb = tsbp.tile([P, 4, P], f32, tag="tsb", name=f"tsb_{c}")
                nc.scalar.copy(tsb.rearrange("p a b -> p (a b)"), aflat)
                outq = attp.tile([P, 4, P], f32, tag="attT", name=f"outq_{c}")
                for k in range(4):
                    nc.tensor.transpose(outq[:, k, :], tsb[:, k, :], ident)
                osb = sbo.tile([P, 4, P], f32, tag="osb", name=f"osb_{c}")
                nc.vector.tensor_copy(
                    osb.rearrange("p a b -> p (a b)"),
                    outq.rearrange("p a b -> p (a b)"),
                )
                nc.sync.dma_start(
                    out=att_d[c * 512 : (c + 1) * 512, :].rearrange(
                        "(t p) v -> p t v", p=P
                    ),
                    in_=osb,
                )

            for c in reversed(range(8)):
                emit_pv_chunk(c)

    nc.compile()
    return nc


def _host_inputs(x, Wq, bq, Wk, bk, Wv, bv):
    """Per-core input maps (host does layout prep: transposes + gathers)."""
    x_full = np.ascontiguousarray(x, dtype=np.float32)
    Wq_s = np.asarray(Wq, np.float32) / 8.0
    wqt = np.ascontiguousarray(np.concatenate([Wq_s.T, Wq_s.T], axis=1))
    bq_s = np.tile((np.asarray(bq, np.float32) / 8.0).reshape(KD, 1), (2, 1))
    WkT_ = np.asarray(Wk, np.float32).T
    wkt = np.ascontiguousarray(np.concatenate([WkT_, WkT_], axis=1))
    bk_ = np.tile(np.asarray(bk, np.float32).reshape(KD, 1), (2, 1))
    wvt = np.ascontiguousarray(np.asarray(Wv, np.float32).T)
    bv_ = np.asarray(bv, np.float32).reshape(VD, 1)

    tri = np.where(
        np.arange(P)[None, :] >= np.arange(P)[:, None], 0.0, -1e9
    ).astype(np.float32)
    mrows = []
    for p in (0, 1):
        m = np.zeros((P, 2 * P), np.float32)
        if p == 0:
            m[:, :P] = tri
        else:
            m[:, :P] = -1e9
            m[:, P:] = tri
        mrows.append(m)

    smalls = [
        np.ascontiguousarray(
            np.concatenate([wqt, wkt, wvt, bq_s, bk_, bv_, mrows[p]], axis=1)
        )
        for p in (0, 1)
    ]
    in_maps = []
    xts = [np.ascontiguousarray(x_full[b].T) for b in range(B)]
    for c in range(NCORES):
        b, p = c // 2, c % 2
        xkvt = np.ascontiguousarray(
            x_full[b].reshape(NT, P, D)[p::2].reshape(JT * P, D).T
        )
        in_maps.append({"xt": xts[b], "xkvt": xkvt, "small": smalls[p]})
    return in_maps


def _get_program():
    if "nc" not in _CACHE:
        _CACHE["nc"] = _build_program()
    return _CACHE["nc"]


def run_on_device(in_maps, trace=False, trace_kwargs=None):
    from concourse import bass_utils

    nc = _get_program()
    return bass_utils.run_bass_kernel_spmd(
        nc,
        in_maps,
        core_ids=list(range(NCORES)),
        trace=trace,
        trace_kwargs=trace_kwargs or {},
    )


def kernel(x, Wq, bq, Wk, bk, Wv, bv):
    x = np.asarray(x, np.float32)
    in_maps = _host_inputs(x, Wq, bq, Wk, bk, Wv, bv)
    res = run_on_device(in_maps)
    att = np.empty((B, S, VD), np.float32)
    for b in range(B):
        att[b] = res.results[2 * b]["att"] + res.results[2 * b + 1]["att"]
    return np.concatenate([x, att], axis=2)



# revision 2
# speedup vs baseline: 1.2718x; 1.2718x over previous
"""Trainium2 Bass kernel for nn_AttentionBlock (column-softmax causal attention).

Reference computation (B=4, S=4096, D=128, K=64, V=128):
    Q = x @ Wq.T + bq            [B,S,64]
    Km = x @ Wk.T + bk           [B,S,64]
    Vm = x @ Wv.T + bv           [B,S,128]
    s  = Q @ Km.T / 8            [B,S,S], causal mask j>q -> -1e9
    p  = softmax(s, axis=1)      (softmax over the QUERY axis -- column softmax)
    att = p @ Vm                 [B,S,128]
    out = concat(x, att, dim=2)  [B,S,256]

Algebraic restructure (lets every matmul run fp16 at 1 cyc/row with full
128-deep contraction):
    s[q,j] = x_q M x_j^T + x_q.a + x_j.b + c   with M = Wq^T Wk / 8,
             a = Wq^T bk / 8, b = Wk^T bq / 8, c = bq.bk / 8.
    The (x_j.b + c) term is constant along the softmax (q) axis for a fixed
    column j, so it CANCELS in softmax(dim=q) and is dropped entirely.
    The x_q.a term folds into G: with G_j = M x_j^T + a (per-partition add),
    s^T[j,q] = sum_d G[d,j] * xT[d,q].
    So: GT = M @ xkv^T + a (tiny matmul), then scores are a single fp16
    128-contraction matmul per (j-tile, q-chunk). No Q/K projections at all.

Flash-style column softmax as in the baseline: E[j,q] = exp(s^T), masked
entries are exp(-1e9)=0; l[j] = sum_q E[j,q] (free-dim reduce);
att^T[v,q] = sum_j (V[j,v]/l[j]) * E[j,q]. Output stays in [v,q] layout --
the HOST transposes (no PE transposes anywhere on device).

Sharding (8 cores): core c -> batch b = c//2, j-tile parity p = c%2.
Host adds the two partial att's per batch.

Pipeline: forward row order (row i = local j-tile i), PV chunk c emitted
after row 2c+1, so the PV matmuls run interleaved with later rows' score
matmuls and the exp stream -- no serial phase C.
"""

import numpy as np

B, S, D = 4, 4096, 128
KD, VD = 64, 128
P = 128
NCORES = 8
JT = 16           # local j-tiles per core
CHUNK = 2048      # score chunk width (PSUM cols)

ROW_W = [S - 2 * i * P for i in range(JT)]          # E row widths
EOFF = [0] * JT
for _i in range(1, JT):
    EOFF[_i] = EOFF[_i - 1] + ROW_W[_i - 1]
ECOLS = EOFF[-1] + ROW_W[-1]                        # 34816

# rows with a single score chunk use the ACT accumulator for l;
# wider rows use a DVE free-dim reduce over the E row instead
ACT_ACCUM_ROWS = [i for i in range(JT) if ROW_W[i] <= CHUNK]

_CACHE = {}


def _build_program():
    from contextlib import ExitStack

    from concourse import bacc, mybir
    from concourse import tile as tile_mod

    dt = mybir.dt
    f32, f16 = dt.float32, dt.float16
    Alu = mybir.AluOpType
    ActF = mybir.ActivationFunctionType

    nc = bacc.Bacc(
        "TRN2", target_bir_lowering=False, debug=False, num_devices=NCORES
    )

    xt_d = nc.dram_tensor("xt", [P, S], f16, kind="ExternalInput").ap()
    xkvt_d = nc.dram_tensor("xkvt", [P, JT * P], f16, kind="ExternalInput").ap()
    # s16: Mt[0:128] | WvT[128:256]
    s16_d = nc.dram_tensor("s16", [P, 256], f16, kind="ExternalInput").ap()
    # s32: a[0] | bvb[1:129] | mrow[129:385]
    s32_d = nc.dram_tensor("s32", [P, 385], f32, kind="ExternalInput").ap()
    att_d = nc.dram_tensor("att", [P, S], f32, kind="ExternalOutput").ap()

    with tile_mod.TileContext(nc) as tc, ExitStack() as ctx:
        persist = ctx.enter_context(tc.tile_pool(name="persist", bufs=1))

        xT = persist.tile([P, S], f16)             # [d, q]
        xkvT = persist.tile([P, JT * P], f16)      # [d, local j]
        GT = persist.tile([P, JT * P], f16)        # [d, local j] = M xkv^T + a
        E_all = persist.tile([P, ECOLS], f16)      # exp(scores^T) rows
        Vp = persist.tile([P, JT, VD], f16)        # [j, v] scaled by 1/l
        l_all = persist.tile([P, JT], f32)
        linv = persist.tile([P, JT], f32)
        lp2 = persist.tile([P, JT], f32)           # chunk-0 l partials (wide rows unused)
        t0 = persist.tile([P, VD], f32)            # V + bv scratch
        s16 = persist.tile([P, 256], f16)
        s32 = persist.tile([P, 385], f32)
        Mt = s16[:, 0:128]
        WvT = s16[:, 128:256]
        a_sb = s32[:, 0:1]
        bvb = s32[:, 1:129]
        mrow = s32[:, 129:385]

        # ---- input DMAs (order per queue = need order)
        nc.gpsimd.dma_start(out=s16, in_=s16_d)
        nc.gpsimd.dma_start(out=s32, in_=s32_d)
        nc.gpsimd.dma_start(out=xkvT[:, 0:1024], in_=xkvt_d[:, 0:1024])
        nc.sync.dma_start(out=xT[:, 0:2048], in_=xt_d[:, 0:2048])
        nc.sync.dma_start(out=xT[:, 2048:4096], in_=xt_d[:, 2048:4096])
        nc.sync.dma_start(out=xkvT[:, 1024:2048], in_=xkvt_d[:, 1024:2048])

        with ExitStack() as ph:
            ps = ph.enter_context(
                tc.tile_pool(name="ps", bufs=2, space="PSUM")
            )
            osb = ph.enter_context(tc.tile_pool(name="osb", bufs=2))

            def emit_gt(g):
                pgt = ps.tile([P, 512], f32, tag="ps", name=f"gt_{g}")
                nc.tensor.matmul(
                    pgt,
                    lhsT=Mt,
                    rhs=xkvT[:, g * 512 : (g + 1) * 512],
                    start=True,
                    stop=True,
                )
                nc.vector.tensor_scalar(
                    out=GT[:, g * 512 : (g + 1) * 512],
                    in0=pgt,
                    scalar1=a_sb,
                    scalar2=None,
                    op0=Alu.add,
                )

            def emit_qk_row(i):
                w = ROW_W[i]
                q0 = 256 * i
                nch = (w + CHUNK - 1) // CHUNK
                for ci in range(nch):
                    cw = min(CHUNK, w - ci * CHUNK)
                    sc = ps.tile([P, CHUNK], f32, tag="ps", name=f"sc_{i}_{ci}")
                    for s0 in range(0, cw, 512):
                        sw = min(512, cw - s0)
                        off = q0 + ci * CHUNK + s0
                        nc.tensor.matmul(
                            sc[:, s0 : s0 + sw],
                            lhsT=GT[:, i * P : (i + 1) * P],
                            rhs=xT[:, off : off + sw],
                            start=True,
                            stop=True,
                        )
                    if ci == 0:
                        nc.vector.tensor_tensor(
                            out=sc[:, : 2 * P],
                            in0=sc[:, : 2 * P],
                            in1=mrow,
                            op=Alu.add,
                        )
                    ecol = EOFF[i] + ci * CHUNK
                    accum = None
                    if i in ACT_ACCUM_ROWS:
                        accum = l_all[:, i : i + 1]
                    elif ci == 0:
                        accum = lp2[:, i : i + 1]
                    else:
                        accum = l_all[:, i : i + 1]
                    nc.scalar.activation(
                        out=E_all[:, ecol : ecol + cw],
                        in_=sc[:, :cw],
                        func=ActF.Exp,
                        accum_out=accum,
                    )
                if i not in ACT_ACCUM_ROWS:
                    nc.vector.tensor_tensor(
                        out=l_all[:, i : i + 1],
                        in0=l_all[:, i : i + 1],
                        in1=lp2[:, i : i + 1],
                        op=Alu.add,
                    )
                nc.vector.reciprocal(linv[:, i : i + 1], l_all[:, i : i + 1])

            def emit_v(i):
                pv = ps.tile([P, VD], f32, tag="ps", name=f"v_{i}")
                nc.tensor.matmul(
                    pv,
                    lhsT=xkvT[:, i * P : (i + 1) * P],
                    rhs=WvT,
                    start=True,
                    stop=True,
                )
                nc.vector.tensor_tensor(
                    out=t0, in0=pv, in1=bvb, op=Alu.add
                )
                nc.vector.tensor_scalar(
                    out=Vp[:, i, :],
                    in0=t0,
                    scalar1=linv[:, i : i + 1],
                    scalar2=None,
                    op0=Alu.mult,
                )

            def emit_pv(c):
                ap = ps.tile([P, 512], f32, tag="ps", name=f"att_{c}")
                for ii in range(2 * c + 1):
                    ecol = EOFF[ii] + 512 * c - 256 * ii
                    nc.tensor.matmul(
                        ap,
                        lhsT=Vp[:, ii, :],
                        rhs=E_all[:, ecol : ecol + 512],
                        start=(ii == 0),
                        stop=False,
                    )
                i2 = 2 * c + 1
                nc.tensor.matmul(
                    ap[:, 256:512],
                    lhsT=Vp[:, i2, :],
                    rhs=E_all[:, EOFF[i2] : EOFF[i2] + 256],
                    start=False,
                    stop=True,
                )
                ob = osb.tile([P, 512], f32, tag="osb", name=f"osb_{c}")
                nc.vector.tensor_copy(ob, ap)
                nc.sync.dma_start(
                    out=att_d[:, c * 512 : (c + 1) * 512], in_=ob
                )

            emit_gt(0)
            emit_gt(1)
            for i in range(JT):
                if i == 6:
                    emit_gt(2)
                if i == 10:
                    emit_gt(3)
                emit_qk_row(i)
                emit_v(i)
                if i % 2 == 1:
                    emit_pv((i - 1) // 2)

    nc.compile()
    return nc


def _host_inputs(x, Wq, bq, Wk, bk, Wv, bv):
    """Per-core input maps (host does layout prep + tiny precomputes)."""
    x_full = np.ascontiguousarray(x, dtype=np.float32)
    Wq = np.asarray(Wq, np.float32)
    Wk = np.asarray(Wk, np.float32)
    bk = np.asarray(bk, np.float32)
    Wv = np.asarray(Wv, np.float32)
    bv = np.asarray(bv, np.float32)

    M = (Wq.T @ Wk) / 8.0                      # [D, D]
    Mt = np.ascontiguousarray(M.T).astype(np.float16)
    a = ((Wq.T @ bk) / 8.0).reshape(D, 1)      # [D, 1]
    WvT = np.ascontiguousarray(Wv.T).astype(np.float16)
    bvb = np.tile(bv.reshape(1, VD), (P, 1))   # [P, V]

    tri = np.where(
        np.arange(P)[None, :] >= np.arange(P)[:, None], 0.0, -1e9
    ).astype(np.float32)
    mrows = []
    for p in (0, 1):
        m = np.zeros((P, 2 * P), np.float32)
        if p == 0:
            m[:, :P] = tri
        else:
            m[:, :P] = -1e9
            m[:, P:] = tri
        mrows.append(m)

    s16 = np.ascontiguousarray(np.concatenate([Mt, WvT], axis=1))
    s32s = [
        np.ascontiguousarray(
            np.concatenate([a, bvb, mrows[p]], axis=1).astype(np.float32)
        )
        for p in (0, 1)
    ]
    xts = [
        np.ascontiguousarray(x_full[b].T.astype(np.float16)) for b in range(B)
    ]
    in_maps = []
    for c in range(NCORES):
        b, p = c // 2, c % 2
        xkvt = np.ascontiguousarray(
            x_full[b].reshape(S // P, P, D)[p::2].reshape(JT * P, D).T
        ).astype(np.float16)
        in_maps.append(
            {"xt": xts[b], "xkvt": xkvt, "s16": s16, "s32": s32s[p]}
        )
    return in_maps


def _get_program():
    if "nc" not in _CACHE:
        _CACHE["nc"] = _build_program()
    return _CACHE["nc"]


def run_on_device(in_maps, trace=False, trace_kwargs=None):
    from concourse import bass_utils

    nc = _get_program()
    return bass_utils.run_bass_kernel_spmd(
        nc,
        in_maps,
        core_ids=list(range(NCORES)),
        trace=trace,
        trace_kwargs=trace_kwargs or {},
    )


def kernel(x, Wq, bq, Wk, bk, Wv, bv):
    x = np.asarray(x, np.float32)
    in_maps = _host_inputs(x, Wq, bq, Wk, bk, Wv, bv)
    res = run_on_device(in_maps)
    out = np.empty((B, S, D + VD), np.float32)
    for b in range(B):
        attT = res.results[2 * b]["att"] + res.results[2 * b + 1]["att"]
        out[b, :, :D] = x[b]
        out[b, :, D:] = attT.T
    return out


# revision 14
# speedup vs baseline: 1.2744x; 1.0020x over previous
"""Trainium2 Bass kernel for nn_AttentionBlock (column-softmax causal attention).

Reference computation (B=4, S=4096, D=128, K=64, V=128):
    Q = x @ Wq.T + bq            [B,S,64]
    Km = x @ Wk.T + bk           [B,S,64]
    Vm = x @ Wv.T + bv           [B,S,128]
    s  = Q @ Km.T / 8            [B,S,S], causal mask j>q -> -1e9
    p  = softmax(s, axis=1)      (softmax over the QUERY axis -- column softmax)
    att = p @ Vm                 [B,S,128]
    out = concat(x, att, dim=2)  [B,S,256]

Algebraic restructure (lets every matmul run fp16 at 1 cyc/row with full
128-deep contraction):
    s[q,j] = x_q M x_j^T + x_q.a + x_j.b + c   with M = Wq^T Wk / 8,
             a = Wq^T bk / 8, b = Wk^T bq / 8, c = bq.bk / 8.
    The (x_j.b + c) term is constant along the softmax (q) axis for a fixed
    column j, so it CANCELS in softmax(dim=q) and is dropped entirely.
    The x_q.a term folds into G: with G_j = M x_j^T + a (per-partition add),
    s^T[j,q] = sum_d G[d,j] * xT[d,q].
    So: GT = M @ xkv^T + a (tiny matmul), then scores are a single fp16
    128-contraction matmul per (j-tile, q-chunk). No Q/K projections at all.

Flash-style column softmax as in the baseline: E[j,q] = exp(s^T), masked
entries are exp(-1e9)=0; l[j] = sum_q E[j,q] (free-dim reduce);
att^T[v,q] = sum_j (V[j,v]/l[j]) * E[j,q]. Output stays in [v,q] layout --
the HOST transposes (no PE transposes anywhere on device).

Sharding (8 cores): core c -> batch b = c//2, j-tile parity p = c%2.
Host adds the two partial att's per batch.

Pipeline: forward row order (row i = local j-tile i), PV chunk c emitted
after row 2c+1, so the PV matmuls run interleaved with later rows' score
matmuls and the exp stream -- no serial phase C.
"""

import numpy as np

B, S, D = 4, 4096, 128
KD, VD = 64, 128
P = 128
NCORES = 8
JT = 16           # local j-tiles per core
CHUNK = 2048      # score chunk width (PSUM cols)

ROW_W = [S - 2 * i * P for i in range(JT)]          # E row widths
EOFF = [0] * JT
for _i in range(1, JT):
    EOFF[_i] = EOFF[_i - 1] + ROW_W[_i - 1]
ECOLS = EOFF[-1] + ROW_W[-1]                        # 34816

# rows with a single score chunk use the ACT accumulator for l;
# wider rows use a DVE free-dim reduce over the E row instead
ACT_ACCUM_ROWS = [i for i in range(JT) if ROW_W[i] <= CHUNK]

_CACHE = {}


def _build_program():
    from contextlib import ExitStack

    from concourse import bacc, mybir
    from concourse import tile as tile_mod

    dt = mybir.dt
    f32, bf16 = dt.float32, dt.bfloat16
    Alu = mybir.AluOpType
    ActF = mybir.ActivationFunctionType

    nc = bacc.Bacc(
        "TRN2", target_bir_lowering=False, debug=False, num_devices=NCORES
    )

    xt_d = nc.dram_tensor("xt", [P, S], bf16, kind="ExternalInput").ap()
    xkvt_d = nc.dram_tensor("xkvt", [P, JT * P], bf16, kind="ExternalInput").ap()
    # s16: Mt[0:128] | WvT[128:256]
    s16_d = nc.dram_tensor("s16", [P, 256], bf16, kind="ExternalInput").ap()
    # s32: a[0] | bvb[1:129] | mrow[129:385]
    s32_d = nc.dram_tensor("s32", [P, 385], f32, kind="ExternalInput").ap()
    att_d = nc.dram_tensor("att", [P, S], f32, kind="ExternalOutput").ap()

    with tile_mod.TileContext(nc) as tc, ExitStack() as ctx:
        persist = ctx.enter_context(tc.tile_pool(name="persist", bufs=1))

        xT = persist.tile([P, S], bf16)            # [d, q]
        xkvT = persist.tile([P, JT * P], bf16)     # [d, local j]
        GT = persist.tile([P, JT * P], bf16)       # [d, local j] = M xkv^T + a
        E_all = persist.tile([P, ECOLS], bf16)     # exp(scores^T) rows
        Vp = persist.tile([P, JT, VD], bf16)       # [j, v] scaled by 1/l
        l_all = persist.tile([P, JT], f32)
        linv = persist.tile([P, JT], f32)
        lp2 = persist.tile([P, JT], f32)           # chunk-1 l partials
        t0 = persist.tile([P, VD], f32)            # V + bv scratch
        o7a = persist.tile([P, 512], f32)          # PV chunk-7 early partial
        warm = persist.tile([P, 8], f32)           # exp-table warmup scratch
        s16 = persist.tile([P, 256], bf16)
        s32 = persist.tile([P, 385], f32)
        Mt = s16[:, 0:128]
        WvT = s16[:, 128:256]
        a_sb = s32[:, 0:1]
        bvb = s32[:, 1:129]
        mrow = s32[:, 129:385]

        # ---- input DMAs spread over four queues (order per queue = need order)
        nc.gpsimd.dma_start(out=s16, in_=s16_d)
        nc.gpsimd.dma_start(out=xkvT[:, 0:512], in_=xkvt_d[:, 0:512])
        nc.gpsimd.dma_start(out=xkvT[:, 512:1024], in_=xkvt_d[:, 512:1024])
        nc.scalar.dma_start(out=s32, in_=s32_d)
        nc.scalar.dma_start(out=xT[:, 1024:2048], in_=xt_d[:, 1024:2048])
        nc.scalar.dma_start(out=xT[:, 2048:3072], in_=xt_d[:, 2048:3072])
        nc.sync.dma_start(out=xT[:, 0:1024], in_=xt_d[:, 0:1024])
        nc.sync.dma_start(out=xT[:, 3072:4096], in_=xt_d[:, 3072:4096])
        nc.sync.dma_start(out=xkvT[:, 1024:2048], in_=xkvt_d[:, 1024:2048])

        # load the EXP activation table while DMAs land
        nc.gpsimd.memset(warm, 0.0)
        nc.scalar.activation(
            out=warm, in_=warm, func=ActF.Exp
        )

        with ExitStack() as ph:
            ps = ph.enter_context(
                tc.tile_pool(name="ps", bufs=2, space="PSUM")
            )
            osb = ph.enter_context(tc.tile_pool(name="osb", bufs=2))

            def emit_gt(g):
                pgt = ps.tile([P, 512], f32, tag="ps", name=f"gt_{g}")
                nc.tensor.matmul(
                    pgt,
                    lhsT=Mt,
                    rhs=xkvT[:, g * 512 : (g + 1) * 512],
                    start=True,
                    stop=True,
                )
                nc.vector.tensor_scalar(
                    out=GT[:, g * 512 : (g + 1) * 512],
                    in0=pgt,
                    scalar1=a_sb,
                    scalar2=None,
                    op0=Alu.add,
                )

            def emit_qk_row(i):
                w = ROW_W[i]
                q0 = 256 * i
                nch = (w + CHUNK - 1) // CHUNK
                for ci in range(nch):
                    cw = min(CHUNK, w - ci * CHUNK)
                    sc = ps.tile([P, CHUNK], f32, tag="ps", name=f"sc_{i}_{ci}")
                    for s0 in range(0, cw, 512):
                        sw = min(512, cw - s0)
                        off = q0 + ci * CHUNK + s0
                        nc.tensor.matmul(
                            sc[:, s0 : s0 + sw],
                            lhsT=GT[:, i * P : (i + 1) * P],
                            rhs=xT[:, off : off + sw],
                            start=True,
                            stop=True,
                        )
                        if ci == 0 and s0 == 0:
                            # mask the diagonal block as soon as its slice
                            # lands (overlaps the remaining slice matmuls)
                            nc.vector.tensor_tensor(
                                out=sc[:, : 2 * P],
                                in0=sc[:, : 2 * P],
                                in1=mrow,
                                op=Alu.add,
                            )
                    ecol = EOFF[i] + ci * CHUNK
                    nc.scalar.activation(
                        out=E_all[:, ecol : ecol + cw],
                        in_=sc[:, :cw],
                        func=ActF.Exp,
                        accum_out=(
                            l_all[:, i : i + 1] if ci == 0 else lp2[:, i : i + 1]
                        ),
                    )
                if nch > 1:
                    nc.vector.tensor_tensor(
                        out=l_all[:, i : i + 1],
                        in0=l_all[:, i : i + 1],
                        in1=lp2[:, i : i + 1],
                        op=Alu.add,
                    )
                nc.vector.reciprocal(linv[:, i : i + 1], l_all[:, i : i + 1])

            def emit_v(i):
                pv = ps.tile([P, VD], f32, tag="ps", name=f"v_{i}")
                nc.tensor.matmul(
                    pv,
                    lhsT=xkvT[:, i * P : (i + 1) * P],
                    rhs=WvT,
                    start=True,
                    stop=True,
                )
                nc.vector.tensor_tensor(
                    out=t0, in0=pv, in1=bvb, op=Alu.add
                )
                nc.vector.tensor_scalar(
                    out=Vp[:, i, :],
                    in0=t0,
                    scalar1=linv[:, i : i + 1],
                    scalar2=None,
                    op0=Alu.mult,
                )

            def emit_pv(c, lo=0, hi=None, merge=None, out_sb=None):
                # att^T chunk c over full-width rows [lo, hi); when hi is
                # None also the half-coverage row 2c+1 closes the group
                tail = hi is None
                hi2 = 2 * c + 1 if tail else hi
                ap = ps.tile([P, 512], f32, tag="ps", name=f"att_{c}_{lo}")
                for ii in range(lo, hi2):
                    ecol = EOFF[ii] + 512 * c - 256 * ii
                    nc.tensor.matmul(
                        ap,
                        lhsT=Vp[:, ii, :],
                        rhs=E_all[:, ecol : ecol + 512],
                        start=(ii == lo),
                        stop=(not tail and ii == hi2 - 1),
                    )
                if tail:
                    i2 = 2 * c + 1
                    nc.tensor.matmul(
                        ap[:, 256:512],
                        lhsT=Vp[:, i2, :],
                        rhs=E_all[:, EOFF[i2] : EOFF[i2] + 256],
                        start=False,
                        stop=True,
                    )
                if out_sb is not None:
                    nc.vector.tensor_copy(out_sb, ap)
                    return
                ob = osb.tile([P, 512], f32, tag="osb", name=f"osb_{c}")
                if merge is None:
                    nc.vector.tensor_copy(ob, ap)
                else:
                    nc.vector.tensor_tensor(
                        out=ob, in0=ap, in1=merge, op=Alu.add
                    )
                nc.sync.dma_start(
                    out=att_d[:, c * 512 : (c + 1) * 512], in_=ob
                )

            emit_gt(0)
            emit_gt(1)
            for i in range(JT):
                if i == 6:
                    emit_gt(2)
                if i == 10:
                    emit_gt(3)
                emit_qk_row(i)
                emit_v(i)
                if i % 2 == 1 and i != 15:
                    emit_pv((i - 1) // 2)
                if i == 11:
                    # early partial of the last PV chunk (rows 0-11): runs
                    # during rows 12-15 so the post-exp tail is tiny
                    emit_pv(7, lo=0, hi=12, out_sb=o7a)
            emit_pv(7, lo=12, merge=o7a)

    nc.compile()
    return nc


def _host_inputs(x, Wq, bq, Wk, bk, Wv, bv):
    """Per-core input maps (host does layout prep + tiny precomputes)."""
    import ml_dtypes

    hf = ml_dtypes.bfloat16
    x_full = np.ascontiguousarray(x, dtype=np.float32)
    Wq = np.asarray(Wq, np.float32)
    Wk = np.asarray(Wk, np.float32)
    bk = np.asarray(bk, np.float32)
    Wv = np.asarray(Wv, np.float32)
    bv = np.asarray(bv, np.float32)

    M = (Wq.T @ Wk) / 8.0                      # [D, D]
    Mt = np.ascontiguousarray(M.T).astype(hf)
    a = ((Wq.T @ bk) / 8.0).reshape(D, 1)      # [D, 1]
    WvT = np.ascontiguousarray(Wv.T).astype(hf)
    bvb = np.tile(bv.reshape(1, VD), (P, 1))   # [P, V]

    tri = np.where(
        np.arange(P)[None, :] >= np.arange(P)[:, None], 0.0, -1e9
    ).astype(np.float32)
    mrows = []
    for p in (0, 1):
        m = np.zeros((P, 2 * P), np.float32)
        if p == 0:
            m[:, :P] = tri
        else:
            m[:, :P] = -1e9
            m[:, P:] = tri
        mrows.append(m)

    s16 = np.ascontiguousarray(np.concatenate([Mt, WvT], axis=1))
    s32s = [
        np.ascontiguousarray(
            np.concatenate([a, bvb, mrows[p]], axis=1).astype(np.float32)
        )
        for p in (0, 1)
    ]
    xts = [
        np.ascontiguousarray(x_full[b].T.astype(hf)) for b in range(B)
    ]
    in_maps = []
    for c in range(NCORES):
        b, p = c // 2, c % 2
        xkvt = np.ascontiguousarray(
            x_full[b].reshape(S // P, P, D)[p::2].reshape(JT * P, D).T
        ).astype(hf)
        in_maps.append(
            {"xt": xts[b], "xkvt": xkvt, "s16": s16, "s32": s32s[p]}
        )
    return in_maps


def _get_program():
    if "nc" not in _CACHE:
        _CACHE["nc"] = _build_program()
    return _CACHE["nc"]


def run_on_device(in_maps, trace=False, trace_kwargs=None):
    from concourse import bass_utils

    nc = _get_program()
    return bass_utils.run_bass_kernel_spmd(
        nc,
        in_maps,
        core_ids=list(range(NCORES)),
        trace=trace,
        trace_kwargs=trace_kwargs or {},
    )


def kernel(x, Wq, bq, Wk, bk, Wv, bv):
    x = np.asarray(x, np.float32)
    in_maps = _host_inputs(x, Wq, bq, Wk, bk, Wv, bv)
    res = run_on_device(in_maps)
    out = np.empty((B, S, D + VD), np.float32)
    for b in range(B):
        attT = res.results[2 * b]["att"] + res.results[2 * b + 1]["att"]
        out[b, :, :D] = x[b]
        out[b, :, D:] = attT.T
    return out


# revision 21
# speedup vs baseline: 1.7071x; 1.3396x over previous
"""Trainium2 Bass kernel for nn_AttentionBlock (column-softmax causal attention).

Reference computation (B=4, S=4096, D=128, K=64, V=128):
    Q = x @ Wq.T + bq            [B,S,64]
    Km = x @ Wk.T + bk           [B,S,64]
    Vm = x @ Wv.T + bv           [B,S,128]
    s  = Q @ Km.T / 8            [B,S,S], causal mask j>q -> -1e9
    p  = softmax(s, axis=1)      (softmax over the QUERY axis -- column softmax)
    att = p @ Vm                 [B,S,128]
    out = concat(x, att, dim=2)  [B,S,256]

Algebraic restructure (lets every matmul run fp16 at 1 cyc/row with full
128-deep contraction):
    s[q,j] = x_q M x_j^T + x_q.a + x_j.b + c   with M = Wq^T Wk / 8,
             a = Wq^T bk / 8, b = Wk^T bq / 8, c = bq.bk / 8.
    The (x_j.b + c) term is constant along the softmax (q) axis for a fixed
    column j, so it CANCELS in softmax(dim=q) and is dropped entirely.
    The x_q.a term folds into G: with G_j = M x_j^T + a (per-partition add),
    s^T[j,q] = sum_d G[d,j] * xT[d,q].
    So: GT = M @ xkv^T + a (tiny matmul), then scores are a single fp16
    128-contraction matmul per (j-tile, q-chunk). No Q/K projections at all.

Flash-style column softmax as in the baseline: E[j,q] = exp(s^T), masked
entries are exp(-1e9)=0; l[j] = sum_q E[j,q] (free-dim reduce);
att^T[v,q] = sum_j (V[j,v]/l[j]) * E[j,q]. Output stays in [v,q] layout --
the HOST transposes (no PE transposes anywhere on device).

Sharding (8 cores): core c -> batch b = c//2, j-tile parity p = c%2.
Host adds the two partial att's per batch.

Pipeline: forward row order (row i = local j-tile i), PV chunk c emitted
after row 2c+1, so the PV matmuls run interleaved with later rows' score
matmuls and the exp stream -- no serial phase C.
"""

import numpy as np

B, S, D = 4, 4096, 128
KD, VD = 64, 128
P = 128
NCORES = 8
JT = 16           # local j-tiles per core
CHUNK = 1536      # score chunk width (PSUM cols, 3 banks)

ROW_W = [S - 2 * i * P for i in range(JT)]          # E row widths
EOFF = [0] * JT
for _i in range(1, JT):
    EOFF[_i] = EOFF[_i - 1] + ROW_W[_i - 1]
ECOLS = EOFF[-1] + ROW_W[-1]                        # 34816

_CACHE = {}


def _build_program():
    from contextlib import ExitStack

    from concourse import bacc, mybir
    from concourse import tile as tile_mod

    dt = mybir.dt
    f32, bf16 = dt.float32, dt.bfloat16
    Alu = mybir.AluOpType
    ActF = mybir.ActivationFunctionType

    nc = bacc.Bacc(
        "TRN2", target_bir_lowering=False, debug=False, num_devices=NCORES
    )

    xt_d = nc.dram_tensor("xt", [P, S], bf16, kind="ExternalInput").ap()
    xkvt_d = nc.dram_tensor("xkvt", [P, JT * P], bf16, kind="ExternalInput").ap()
    # s16: Mt[0:128] | WvT[128:256]
    s16_d = nc.dram_tensor("s16", [P, 256], bf16, kind="ExternalInput").ap()
    # s32: a[0] | bvb[1:129] | mrow[129:385]
    s32_d = nc.dram_tensor("s32", [P, 385], f32, kind="ExternalInput").ap()
    att_d = nc.dram_tensor("att", [P, S], f32, kind="ExternalOutput").ap()

    with tile_mod.TileContext(nc) as tc, ExitStack() as ctx:
        persist = ctx.enter_context(tc.tile_pool(name="persist", bufs=1))

        xT = persist.tile([P, S], bf16)            # [d, q]
        xkvT = persist.tile([P, JT * P], bf16)     # [d, local j]
        GT = persist.tile([P, JT * P], bf16)       # [d, local j] = M xkv^T + a
        E_all = persist.tile([P, ECOLS], bf16)     # exp(scores^T) rows
        Vp = persist.tile([P, JT, VD], bf16)       # [j, v] scaled by 1/l
        l_all = persist.tile([P, JT], f32)
        linv = persist.tile([P, JT], f32)
        lp2 = persist.tile([P, JT], f32)           # chunk-1 l partials
        lp3 = persist.tile([P, JT], f32)           # chunk-2 l partials
        V_sb = persist.tile([P, JT, VD], f32)      # V + bv, unscaled
        o7a = persist.tile([P, 512], f32)          # PV chunk-7 early partial
        warm = persist.tile([P, 8], f32)           # exp-table warmup scratch
        s16 = persist.tile([P, 256], bf16)
        s32 = persist.tile([P, 385], f32)
        Mt = s16[:, 0:128]
        WvT = s16[:, 128:256]
        a_sb = s32[:, 0:1]
        bvb = s32[:, 1:129]
        mrow = s32[:, 129:385]

        # ---- input DMAs: critical pieces ride the HWDGE queues (sync/
        # scalar); the SWDGE gpsimd queue gets one non-urgent piece
        nc.sync.dma_start(out=s16, in_=s16_d)
        nc.sync.dma_start(out=xkvT[:, 0:512], in_=xkvt_d[:, 0:512])
        nc.sync.dma_start(out=xT[:, 0:1024], in_=xt_d[:, 0:1024])
        nc.sync.dma_start(out=xkvT[:, 1024:2048], in_=xkvt_d[:, 1024:2048])
        nc.sync.dma_start(out=xT[:, 3072:4096], in_=xt_d[:, 3072:4096])
        nc.scalar.dma_start(out=s32, in_=s32_d)
        nc.scalar.dma_start(out=xT[:, 1024:2048], in_=xt_d[:, 1024:2048])
        nc.scalar.dma_start(out=xT[:, 2048:3072], in_=xt_d[:, 2048:3072])
        nc.gpsimd.dma_start(out=xkvT[:, 512:1024], in_=xkvt_d[:, 512:1024])

        # load the EXP activation table while DMAs land
        nc.gpsimd.memset(warm, 0.0)
        nc.scalar.activation(
            out=warm, in_=warm, func=ActF.Exp
        )

        with ExitStack() as ph:
            ps = ph.enter_context(
                tc.tile_pool(name="ps", bufs=2, space="PSUM")
            )
            aux = ph.enter_context(
                tc.tile_pool(name="aux", bufs=2, space="PSUM")
            )
            osb = ph.enter_context(tc.tile_pool(name="osb", bufs=2))

            def emit_gt(g):
                pgt = aux.tile([P, 512], f32, tag="aux", name=f"gt_{g}")
                nc.tensor.matmul(
                    pgt,
                    lhsT=Mt,
                    rhs=xkvT[:, g * 512 : (g + 1) * 512],
                    start=True,
                    stop=True,
                )
                nc.vector.tensor_scalar(
                    out=GT[:, g * 512 : (g + 1) * 512],
                    in0=pgt,
                    scalar1=a_sb,
                    scalar2=None,
                    op0=Alu.add,
                )

            def emit_qk_row(i):
                w = ROW_W[i]
                q0 = 256 * i
                nch = (w + CHUNK - 1) // CHUNK
                for ci in range(nch):
                    cw = min(CHUNK, w - ci * CHUNK)
                    sc = ps.tile([P, CHUNK], f32, tag="ps", name=f"sc_{i}_{ci}")
                    for s0 in range(0, cw, 512):
                        sw = min(512, cw - s0)
                        off = q0 + ci * CHUNK + s0
                        nc.tensor.matmul(
                            sc[:, s0 : s0 + sw],
                            lhsT=GT[:, i * P : (i + 1) * P],
                            rhs=xT[:, off : off + sw],
                            start=True,
                            stop=True,
                        )
                        if ci == 0 and s0 == 0:
                            # mask the diagonal block as soon as its slice
                            # lands (overlaps the remaining slice matmuls)
                            nc.vector.tensor_tensor(
                                out=sc[:, : 2 * P],
                                in0=sc[:, : 2 * P],
                                in1=mrow,
                                op=Alu.add,
                            )
                    ecol = EOFF[i] + ci * CHUNK
                    nc.scalar.activation(
                        out=E_all[:, ecol : ecol + cw],
                        in_=sc[:, :cw],
                        func=ActF.Exp,
                        accum_out=[l_all, lp2, lp3][ci][:, i : i + 1],
                    )
                for pp in ([lp2, lp3][: nch - 1]):
                    nc.vector.tensor_tensor(
                        out=l_all[:, i : i + 1],
                        in0=l_all[:, i : i + 1],
                        in1=pp[:, i : i + 1],
                        op=Alu.add,
                    )
                nc.vector.reciprocal(linv[:, i : i + 1], l_all[:, i : i + 1])
                nc.vector.tensor_scalar(
                    out=Vp[:, i, :],
                    in0=V_sb[:, i, :],
                    scalar1=linv[:, i : i + 1],
                    scalar2=None,
                    op0=Alu.mult,
                )

            def emit_v(i):
                # V projection for tile i (runs at startup; needs no l)
                pv = aux.tile([P, VD], f32, tag="aux", name=f"v_{i}")
                nc.tensor.matmul(
                    pv,
                    lhsT=xkvT[:, i * P : (i + 1) * P],
                    rhs=WvT,
                    start=True,
                    stop=True,
                )
                nc.vector.tensor_tensor(
                    out=V_sb[:, i, :], in0=pv, in1=bvb, op=Alu.add
                )

            def emit_pv(c, lo=0, hi=None, merge=None, out_sb=None):
                # att^T chunk c over full-width rows [lo, hi); when hi is
                # None also the half-coverage row 2c+1 closes the group
                tail = hi is None
                hi2 = 2 * c + 1 if tail else hi
                ap = aux.tile([P, 512], f32, tag="aux", name=f"att_{c}_{lo}")
                for ii in range(lo, hi2):
                    ecol = EOFF[ii] + 512 * c - 256 * ii
                    nc.tensor.matmul(
                        ap,
                        lhsT=Vp[:, ii, :],
                        rhs=E_all[:, ecol : ecol + 512],
                        start=(ii == lo),
                        stop=(not tail and ii == hi2 - 1),
                    )
                if tail:
                    i2 = 2 * c + 1
                    nc.tensor.matmul(
                        ap[:, 256:512],
                        lhsT=Vp[:, i2, :],
                        rhs=E_all[:, EOFF[i2] : EOFF[i2] + 256],
                        start=False,
                        stop=True,
                    )
                if out_sb is not None:
                    nc.vector.tensor_copy(out_sb, ap)
                    return
                ob = osb.tile([P, 512], f32, tag="osb", name=f"osb_{c}")
                if merge is None:
                    nc.vector.tensor_copy(ob, ap)
                else:
                    nc.vector.tensor_tensor(
                        out=ob, in0=ap, in1=merge, op=Alu.add
                    )
                nc.sync.dma_start(
                    out=att_d[:, c * 512 : (c + 1) * 512], in_=ob
                )

            emit_gt(0)
            for t in range(0, 4):
                emit_v(t)
            emit_gt(1)
            for t in range(4, 8):
                emit_v(t)
            for i in range(JT):
                if i == 2:
                    emit_gt(2)
                    for t in range(8, 12):
                        emit_v(t)
                if i == 4:
                    emit_gt(3)
                    for t in range(12, 16):
                        emit_v(t)
                emit_qk_row(i)
                if i % 2 == 1 and i != 15:
                    emit_pv((i - 1) // 2)
                if i == 11:
                    # early partial of the last PV chunk (rows 0-11): runs
                    # during rows 12-15 so the post-exp tail is tiny
                    emit_pv(7, lo=0, hi=12, out_sb=o7a)
            emit_pv(7, lo=12, merge=o7a)

    nc.compile()
    return nc


def _host_inputs(x, Wq, bq, Wk, bk, Wv, bv):
    """Per-core input maps (host does layout prep + tiny precomputes)."""
    import ml_dtypes

    hf = ml_dtypes.bfloat16
    x_full = np.ascontiguousarray(x, dtype=np.float32)
    Wq = np.asarray(Wq, np.float32)
    Wk = np.asarray(Wk, np.float32)
    bk = np.asarray(bk, np.float32)
    Wv = np.asarray(Wv, np.float32)
    bv = np.asarray(bv, np.float32)

    M = (Wq.T @ Wk) / 8.0                      # [D, D]
    Mt = np.ascontiguousarray(M.T).astype(hf)
    a = ((Wq.T @ bk) / 8.0).reshape(D, 1)      # [D, 1]
    WvT = np.ascontiguousarray(Wv.T).astype(hf)
    bvb = np.tile(bv.reshape(1, VD), (P, 1))   # [P, V]

    tri = np.where(
        np.arange(P)[None, :] >= np.arange(P)[:, None], 0.0, -1e9
    ).astype(np.float32)
    mrows = []
    for p in (0, 1):
        m = np.zeros((P, 2 * P), np.float32)
        if p == 0:
            m[:, :P] = tri
        else:
            m[:, :P] = -1e9
            m[:, P:] = tri
        mrows.append(m)

    s16 = np.ascontiguousarray(np.concatenate([Mt, WvT], axis=1))
    s32s = [
        np.ascontiguousarray(
            np.concatenate([a, bvb, mrows[p]], axis=1).astype(np.float32)
        )
        for p in (0, 1)
    ]
    xts = [
        np.ascontiguousarray(x_full[b].T.astype(hf)) for b in range(B)
    ]
    in_maps = []
    for c in range(NCORES):
        b, p = c // 2, c % 2
        xkvt = np.ascontiguousarray(
            x_full[b].reshape(S // P, P, D)[p::2].reshape(JT * P, D).T
        ).astype(hf)
        in_maps.append(
            {"xt": xts[b], "xkvt": xkvt, "s16": s16, "s32": s32s[p]}
        )
    return in_maps


def _get_program():
    if "nc" not in _CACHE:
        _CACHE["nc"] = _build_program()
    return _CACHE["nc"]


def run_on_device(in_maps, trace=False, trace_kwargs=None):
    from concourse import bass_utils

    nc = _get_program()
    return bass_utils.run_bass_kernel_spmd(
        nc,
        in_maps,
        core_ids=list(range(NCORES)),
        trace=trace,
        trace_kwargs=trace_kwargs or {},
    )


def kernel(x, Wq, bq, Wk, bk, Wv, bv):
    x = np.asarray(x, np.float32)
    in_maps = _host_inputs(x, Wq, bq, Wk, bk, Wv, bv)
    res = run_on_device(in_maps)
    out = np.empty((B, S, D + VD), np.float32)
    for b in range(B):
        attT = res.results[2 * b]["att"] + res.results[2 * b + 1]["att"]
        out[b, :, :D] = x[b]
        out[b, :, D:] = attT.T
    return out
